# revision 33
# baseline (speedup 1.0000x reference)
"""Trainium2 Bass kernel for nn_Attention_24781961298297.

Math: scores[b,i,j] = (q_term[b,i] + k_term[b,j]) / sqrt(A).  Softmax over j
subtracts the row max, and q_term[b,i] is constant along j, so it cancels
exactly -- the attention weights are independent of i (and of the whole
decoder/q branch).  The output is one [A] vector per batch element,
broadcast over all Ld rows:

    kt[b,j] = relu(enc[b,j] @ Wk) @ (Pu @ pv)      (biases are zero)
    w[b]    = softmax(kt[b] / sqrt(A))
    row[b]  = w[b] @ relu(enc[b] @ Wv)
    out[b,i,:] = row[b]  for all i

Sharding: pure data-parallel over batch B=8 across the 8 cores (one batch
element per core, no collectives).

v7 notes (trace-driven, from six measured variants):
  * Fine-grained 10-chunk pipeline (256/512-token chunks) -- measured
    better wall-clock than every big-chunk variant (latency hiding beats
    per-op overhead savings on this machine).
  * enc ships fp8e4m3 in host-prepared piece-major blocks (8 pieces of
    512 tokens) whose per-partition rows are 2 KB contiguous -> 2 KB DMA
    descriptors.  v1's token-sliced layout gave 512 B descriptors and
    only ~88 GB/s; this layout measured ~300 GB/s aggregate.  Pieces
    alternate between the two HWDGE rings; Wk and Wv are separate 64 KB
    params, one per ring, ordered before the enc pieces.
  * Projections are fp8 DoubleRow matmuls (K=256/instruction, measured
    216 ns back-to-back at full clock, half the bf16 instruction count).
  * e row lives as [1, LE] (no [128, LE] memset -- saves 4.4 us of DVE);
    softmax-weight broadcast is a K=1 ones-matmul; kt is an M=1 matmul
    into a [1, 512] PSUM row.
  * Engine split per chunk: one relu on ACT and one on DVE (alternating
    K/V), exp on ACT, weighted-sum (scalar_tensor_tensor + accum) on
    DVE.  GpSimd cannot help (no PSUM access, no TensorScalar opcode).
  * Pipeline lags: ktp/exp one chunk behind the projections, wb/stt two
    behind -> no PE instruction waits on same-chunk ACT/DVE results.
    PSUM (1 bank each): kps(2) + vps(2) + ktp(2) + wb/warm(2) = 8.
"""

import numpy as np
import ml_dtypes

import concourse.bass as bass
import concourse.bacc as bacc
import concourse.tile as tile
from concourse import mybir
from concourse.bass_utils import run_bass_kernel_spmd

B, LE, LD = 8, 4096, 4096
DE, DD, A = 512, 512, 128

NDC = DE // 128                    # 4 DE subtiles
# one DMA piece per compute chunk; small first pieces so the first
# projection starts as early as possible
SIZES = [256, 256, 512, 512, 512, 512, 512, 512, 256, 256]
NCH = len(SIZES)
OFFS = [sum(SIZES[:i]) for i in range(NCH)]

INV_SQRT_A = float(1.0 / np.sqrt(np.float32(A)))

F32 = mybir.dt.float32
BF16 = mybir.dt.bfloat16
FP8 = mybir.dt.float8e4
Relu = mybir.ActivationFunctionType.Relu
Exp = mybir.ActivationFunctionType.Exp
AX = mybir.AxisListType.X
ADD = mybir.AluOpType.add
MAX = mybir.AluOpType.max
MULT = mybir.AluOpType.mult
BYPASS = mybir.AluOpType.bypass
DR = mybir.MatmulPerfMode.DoubleRow

N_WARM = 3


def build_nc() -> bass.Bass:
    nc = bacc.Bacc()

    enc_ps = [
        nc.declare_dram_parameter(f"enc{t}", [128, NDC * sz], FP8,
                                  isOutput=False)
        for t, sz in enumerate(SIZES)
    ]
    wk = nc.declare_dram_parameter("wk", [128, NDC * A], FP8, isOutput=False)
    wv = nc.declare_dram_parameter("wv", [128, NDC * A], FP8, isOutput=False)
    u_pad = nc.declare_dram_parameter("u_pad", [A, 128], BF16, isOutput=False)
    out = nc.declare_dram_parameter("out", [A, 128], F32, isOutput=True)

    with tile.TileContext(nc) as tc:
        with (
            tc.tile_pool(name="consts", bufs=1) as consts,
            tc.tile_pool(name="encpool", bufs=1) as encpool,
            tc.tile_pool(name="kvp", bufs=1) as kvp,
            tc.tile_pool(name="smallp", bufs=1) as smallp,
            tc.tile_pool(name="work", bufs=2) as work,
            tc.tile_pool(name="ps_k", bufs=2, space="PSUM") as ps_k,
            tc.tile_pool(name="ps_v", bufs=2, space="PSUM") as ps_v,
            tc.tile_pool(name="ps_kt", bufs=2, space="PSUM") as ps_kt,
            tc.tile_pool(name="ps_wb", bufs=2, space="PSUM") as ps_wb,
        ):
            # ---- DMAs split between the sync HWDGE ring and GpSimd's
            #      SWDGE (GpSimd is otherwise idle; configs on the ACT ring
            #      would cost ~667 ns each of ACT-queue time).  Weights
            #      first on each path, then pieces alternating.
            wk_sb = consts.tile([128, NDC, A], FP8, tag="wk")
            wv_sb = consts.tile([128, NDC, A], FP8, tag="wv")
            enc_sb = []
            for t, sz in enumerate(SIZES):
                et = encpool.tile([128, NDC, sz], FP8, tag=f"enc{t}",
                                  name=f"enc_sb{t}")
                enc_sb.append(et)
            up_sb = consts.tile([A, 128], BF16, tag="up")

            def piece_dma(eng, t):
                eng.dma_start(
                    out=enc_sb[t],
                    in_=enc_ps[t].rearrange("p (c j) -> p c j", c=NDC))

            # sync HWDGE: piece 0, Wk, then odd pieces (FIFO = consumption
            # order per path); GpSimd SWDGE: Wv, u, then even pieces --
            # every piece lands well before its chunk is scheduled.
            piece_dma(nc.sync, 0)
            nc.sync.dma_start(out=wk_sb,
                              in_=wk.rearrange("p (c a) -> p c a", c=NDC))
            nc.gpsimd.dma_start(out=wv_sb,
                                in_=wv.rearrange("p (c a) -> p c a", c=NDC))
            piece_dma(nc.sync, 1)
            nc.gpsimd.dma_start(out=up_sb, in_=u_pad[:, :])
            for t in (3, 5, 7, 9):
                piece_dma(nc.sync, t)
            for t in (2, 4, 6, 8):
                piece_dma(nc.gpsimd, t)

            # ---- tiny SBUF constants + PE warm-up
            ones1 = consts.tile([1, 128], BF16, tag="ones1")
            nc.vector.memset(ones1, 1.0)
            wtile = consts.tile([1, 512], BF16, tag="wtile")
            nc.vector.memset(wtile, 0.5)
            for _ in range(N_WARM):
                warm_ps = ps_wb.tile([128, 512], F32, tag="wb")
                nc.tensor.matmul(warm_ps, lhsT=ones1, rhs=wtile,
                                 start=True, stop=True)

            e_sb = smallp.tile([1, LE], BF16, tag="e")
            ssum = smallp.tile([1, NCH], F32, tag="ssum")
            partial = smallp.tile([A, NCH], F32, tag="partial")
            out_pad = smallp.tile([A, 128], F32, tag="out_pad")
            nc.gpsimd.memset(out_pad, 0.0)

            vps_t = {}   # PSUM V-projection per chunk
            ktp_t = {}   # PSUM [1, sz] logits per chunk
            kT_t = {}    # SBUF relu'd K per chunk
            vT_t = {}    # SBUF relu'd V per chunk

            def emit_proj(i, pool, w_sb, tag):
                sz = SIZES[i]
                ps = pool.tile([128, 512], F32, tag=tag)
                for c in range(0, NDC, 2):
                    nc.tensor.matmul(
                        ps[:, :sz], lhsT=w_sb[:, c:c + 2, :],
                        rhs=enc_sb[i][:, c:c + 2, :],
                        start=(c == 0), stop=(c == NDC - 2),
                        perf_mode=DR,
                    )
                return ps

            def relu_op(on_act, dst, src):
                if on_act:
                    nc.scalar.activation(out=dst, in_=src, func=Relu,
                                         bias=0.0, scale=1.0)
                else:
                    nc.vector.tensor_scalar(out=dst, in0=src, scalar1=0.0,
                                            scalar2=None, op0=MAX)

            def emit_relu_k(i, kps):
                sz = SIZES[i]
                kT = kvp.tile([A, 512], BF16, tag="kT", bufs=2)
                relu_op(i % 2 == 1, kT[:, :sz], kps[:, :sz])
                kT_t[i] = kT

            def emit_relu_v(i):
                sz = SIZES[i]
                vT = kvp.tile([A, 512], BF16, tag="vT", bufs=3)
                relu_op(i % 2 == 0 and i not in (4, 6), vT[:, :sz],
                        vps_t[i][:, :sz])
                vT_t[i] = vT
                del vps_t[i]

            def emit_kt(i):
                sz = SIZES[i]
                ktp = ps_kt.tile([1, 512], F32, tag="ktp")
                nc.tensor.matmul(ktp[:, :sz], lhsT=up_sb[:, 0:1],
                                 rhs=kT_t[i][:, :sz], start=True, stop=True)
                ktp_t[i] = ktp
                del kT_t[i]

            def emit_exp(i):
                sz = SIZES[i]
                off = OFFS[i]
                nc.scalar.activation(
                    out=e_sb[0:1, off:off + sz], in_=ktp_t[i][:, :sz],
                    func=Exp, bias=0.0, scale=1.0,
                    accum_out=ssum[:, i:i + 1])
                del ktp_t[i]

            def emit_wb_stt(i):
                sz = SIZES[i]
                off = OFFS[i]
                wb = ps_wb.tile([128, 512], F32, tag="wb")
                nc.tensor.matmul(wb[:, :sz], lhsT=ones1,
                                 rhs=e_sb[0:1, off:off + sz],
                                 start=True, stop=True)
                prod = work.tile([A, 512], BF16, tag="prod")
                nc.vector.scalar_tensor_tensor(
                    out=prod[:, :sz], in0=vT_t[i][:, :sz], scalar=0.0,
                    in1=wb[:, :sz], op0=BYPASS, op1=MULT,
                    accum_out=partial[:, i:i + 1])
                del vT_t[i]

            for i in range(NCH):
                kps = emit_proj(i, ps_k, wk_sb, "kps")
                emit_relu_k(i, kps)
                if i >= 1:
                    emit_kt(i - 1)
                    emit_exp(i - 1)
                vps_t[i] = emit_proj(i, ps_v, wv_sb, "vps")
                emit_relu_v(i)
                if i >= 2:
                    emit_wb_stt(i - 2)
            emit_kt(NCH - 1)
            emit_exp(NCH - 1)
            emit_wb_stt(NCH - 2)
            emit_wb_stt(NCH - 1)

            # ---- unnormalized row + S; host divides and broadcasts.
            nc.vector.reduce_sum(out=out_pad[0:1, 1:2], in_=ssum, axis=AX,
                                 op=ADD)
            nc.vector.reduce_sum(out=out_pad[:, 0:1], in_=partial, axis=AX,
                                 op=ADD)
            nc.sync.dma_start(out=out[:, :], in_=out_pad)

    nc.finalize()
    return nc


def make_in_maps(inputs) -> list[dict]:
    f8 = ml_dtypes.float8_e4m3
    bf16 = ml_dtypes.bfloat16
    enc = np.asarray(inputs["encoder_outputs"], dtype=np.float32)
    Wk = np.asarray(inputs["Wk"], dtype=np.float32)
    Wv = np.asarray(inputs["Wv"], dtype=np.float32)
    Pu = np.asarray(inputs["Pu"], dtype=np.float32)
    pv = np.asarray(inputs["pv"], dtype=np.float32)

    u = (Pu @ pv).astype(np.float32) * INV_SQRT_A          # [A, 1]
    u_pad = np.zeros((A, 128), np.float32)
    u_pad[:, 0:1] = u
    u_pad = u_pad.astype(bf16)

    def wprep(w):  # [DE, A] -> [128, NDC*A], c-major per partition
        return np.ascontiguousarray(
            w.reshape(NDC, 128, A).transpose(1, 0, 2).reshape(128, -1)
        ).astype(f8)

    maps = []
    for b in range(B):
        encT = np.ascontiguousarray(enc[b].T).astype(f8)   # [DE, LE]
        m = {"wk": wprep(Wk), "wv": wprep(Wv), "u_pad": u_pad}
        for t, sz in enumerate(SIZES):
            blk = encT[:, OFFS[t]:OFFS[t] + sz]            # [DE, sz]
            m[f"enc{t}"] = np.ascontiguousarray(
                blk.reshape(NDC, 128, sz).transpose(1, 0, 2)
                .reshape(128, NDC * sz))
        maps.append(m)
    return maps


_NC_CACHE = None


def kernel(**inputs) -> np.ndarray:
    global _NC_CACHE
    in_maps = make_in_maps(inputs)
    if _NC_CACHE is None:
        _NC_CACHE = build_nc()
    res = run_bass_kernel_spmd(_NC_CACHE, in_maps, core_ids=list(range(B)))
    rows = []
    for b in range(B):
        o = np.asarray(res.results[b]["out"], dtype=np.float32)
        rows.append(o[:, 0] / o[0, 1])
    rows = np.stack(rows)                          # [B, A]
    return np.ascontiguousarray(
        np.broadcast_to(rows[:, None, :], (B, LD, A)).astype(np.float32)
    )


# revision 34
# speedup vs baseline: 1.0271x; 1.0271x over previous
"""Trainium2 Bass kernel for nn_Attention_24781961298297.

Math: scores[b,i,j] = (q_term[b,i] + k_term[b,j]) / sqrt(A).  Softmax over j
subtracts the row max, and q_term[b,i] is constant along j, so it cancels
exactly -- the attention weights are independent of i (and of the whole
decoder/q branch).  The output is one [A] vector per batch element,
broadcast over all Ld rows:

    kt[b,j] = relu(enc[b,j] @ Wk) @ (Pu @ pv)      (biases are zero)
    w[b]    = softmax(kt[b] / sqrt(A))
    row[b]  = w[b] @ relu(enc[b] @ Wv)
    out[b,i,:] = row[b]  for all i

Sharding: pure data-parallel over batch B=8 across the 8 cores (one batch
element per core, no collectives).

v7 notes (trace-driven, from six measured variants):
  * Fine-grained 10-chunk pipeline (256/512-token chunks) -- measured
    better wall-clock than every big-chunk variant (latency hiding beats
    per-op overhead savings on this machine).
  * enc ships fp8e4m3 in host-prepared piece-major blocks (8 pieces of
    512 tokens) whose per-partition rows are 2 KB contiguous -> 2 KB DMA
    descriptors.  v1's token-sliced layout gave 512 B descriptors and
    only ~88 GB/s; this layout measured ~300 GB/s aggregate.  Pieces
    alternate between the two HWDGE rings; Wk and Wv are separate 64 KB
    params, one per ring, ordered before the enc pieces.
  * Projections are fp8 DoubleRow matmuls (K=256/instruction, measured
    216 ns back-to-back at full clock, half the bf16 instruction count).
  * e row lives as [1, LE] (no [128, LE] memset -- saves 4.4 us of DVE);
    softmax-weight broadcast is a K=1 ones-matmul; kt is an M=1 matmul
    into a [1, 512] PSUM row.
  * Engine split per chunk: one relu on ACT and one on DVE (alternating
    K/V), exp on ACT, weighted-sum (scalar_tensor_tensor + accum) on
    DVE.  GpSimd cannot help (no PSUM access, no TensorScalar opcode).
  * Pipeline lags: ktp/exp one chunk behind the projections, wb/stt two
    behind -> no PE instruction waits on same-chunk ACT/DVE results.
    PSUM (1 bank each): kps(2) + vps(2) + ktp(2) + wb/warm(2) = 8.
"""

import numpy as np
import ml_dtypes

import concourse.bass as bass
import concourse.bacc as bacc
import concourse.tile as tile
from concourse import mybir
from concourse.bass_utils import run_bass_kernel_spmd

B, LE, LD = 8, 4096, 4096
DE, DD, A = 512, 512, 128

NDC = DE // 128                    # 4 DE subtiles
# one DMA piece per compute chunk; small first pieces so the first
# projection starts as early as possible
SIZES = [256, 256, 512, 512, 512, 512, 512, 512, 256, 256]
NCH = len(SIZES)
OFFS = [sum(SIZES[:i]) for i in range(NCH)]

INV_SQRT_A = float(1.0 / np.sqrt(np.float32(A)))

F32 = mybir.dt.float32
BF16 = mybir.dt.bfloat16
FP8 = mybir.dt.float8e4
Relu = mybir.ActivationFunctionType.Relu
Exp = mybir.ActivationFunctionType.Exp
AX = mybir.AxisListType.X
ADD = mybir.AluOpType.add
MAX = mybir.AluOpType.max
MULT = mybir.AluOpType.mult
BYPASS = mybir.AluOpType.bypass
DR = mybir.MatmulPerfMode.DoubleRow

N_WARM = 3


def build_nc() -> bass.Bass:
    nc = bacc.Bacc()

    enc_ps = [
        nc.declare_dram_parameter(f"enc{t}", [128, NDC * sz], FP8,
                                  isOutput=False)
        for t, sz in enumerate(SIZES)
    ]
    wk = nc.declare_dram_parameter("wk", [128, NDC * A], FP8, isOutput=False)
    wv = nc.declare_dram_parameter("wv", [128, NDC * A], FP8, isOutput=False)
    u_pad = nc.declare_dram_parameter("u_pad", [A, 128], BF16, isOutput=False)
    out = nc.declare_dram_parameter("out", [A, 128], F32, isOutput=True)

    with tile.TileContext(nc) as tc:
        with (
            tc.tile_pool(name="consts", bufs=1) as consts,
            tc.tile_pool(name="encpool", bufs=1) as encpool,
            tc.tile_pool(name="kvp", bufs=1) as kvp,
            tc.tile_pool(name="smallp", bufs=1) as smallp,
            tc.tile_pool(name="work", bufs=2) as work,
            tc.tile_pool(name="ps_k", bufs=2, space="PSUM") as ps_k,
            tc.tile_pool(name="ps_v", bufs=2, space="PSUM") as ps_v,
            tc.tile_pool(name="ps_kt", bufs=2, space="PSUM") as ps_kt,
            tc.tile_pool(name="ps_wb", bufs=2, space="PSUM") as ps_wb,
        ):
            # ---- DMAs split between the sync HWDGE ring and GpSimd's
            #      SWDGE (GpSimd is otherwise idle; configs on the ACT ring
            #      would cost ~667 ns each of ACT-queue time).  Weights
            #      first on each path, then pieces alternating.
            wk_sb = consts.tile([128, NDC, A], FP8, tag="wk")
            wv_sb = consts.tile([128, NDC, A], FP8, tag="wv")
            enc_sb = []
            for t, sz in enumerate(SIZES):
                et = encpool.tile([128, NDC, sz], FP8, tag=f"enc{t}",
                                  name=f"enc_sb{t}")
                enc_sb.append(et)
            up_sb = consts.tile([A, 128], BF16, tag="up")

            def piece_dma(eng, t):
                eng.dma_start(
                    out=enc_sb[t],
                    in_=enc_ps[t].rearrange("p (c j) -> p c j", c=NDC))

            # sync HWDGE: piece 0, Wk, then odd pieces (FIFO = consumption
            # order per path); GpSimd SWDGE: Wv, u, then even pieces --
            # every piece lands well before its chunk is scheduled.
            piece_dma(nc.sync, 0)
            nc.sync.dma_start(out=wk_sb,
                              in_=wk.rearrange("p (c a) -> p c a", c=NDC))
            nc.gpsimd.dma_start(out=wv_sb,
                                in_=wv.rearrange("p (c a) -> p c a", c=NDC))
            piece_dma(nc.sync, 1)
            piece_dma(nc.gpsimd, 2)
            nc.gpsimd.dma_start(out=up_sb, in_=u_pad[:, :])
            for t in (3, 5, 7, 9):
                piece_dma(nc.sync, t)
            for t in (4, 6, 8):
                piece_dma(nc.gpsimd, t)

            # ---- tiny SBUF constants + PE warm-up
            ones1 = consts.tile([1, 128], BF16, tag="ones1")
            nc.vector.memset(ones1, 1.0)
            wtile = consts.tile([1, 512], BF16, tag="wtile")
            nc.vector.memset(wtile, 0.5)
            for _ in range(N_WARM):
                warm_ps = ps_wb.tile([128, 512], F32, tag="wb")
                nc.tensor.matmul(warm_ps, lhsT=ones1, rhs=wtile,
                                 start=True, stop=True)

            e_sb = smallp.tile([1, LE], BF16, tag="e")
            ssum = smallp.tile([1, NCH], F32, tag="ssum")
            partial = smallp.tile([A, NCH], F32, tag="partial")
            out_pad = smallp.tile([A, 128], F32, tag="out_pad")
            nc.gpsimd.memset(out_pad, 0.0)

            vps_t = {}   # PSUM V-projection per chunk
            ktp_t = {}   # PSUM [1, sz] logits per chunk
            kT_t = {}    # SBUF relu'd K per chunk
            vT_t = {}    # SBUF relu'd V per chunk

            def emit_proj(i, pool, w_sb, tag):
                sz = SIZES[i]
                ps = pool.tile([128, 512], F32, tag=tag)
                for c in range(0, NDC, 2):
                    nc.tensor.matmul(
                        ps[:, :sz], lhsT=w_sb[:, c:c + 2, :],
                        rhs=enc_sb[i][:, c:c + 2, :],
                        start=(c == 0), stop=(c == NDC - 2),
                        perf_mode=DR,
                    )
                return ps

            def relu_op(on_act, dst, src):
                if on_act:
                    nc.scalar.activation(out=dst, in_=src, func=Relu,
                                         bias=0.0, scale=1.0)
                else:
                    nc.vector.tensor_scalar(out=dst, in0=src, scalar1=0.0,
                                            scalar2=None, op0=MAX)

            def emit_relu_k(i, kps):
                sz = SIZES[i]
                kT = kvp.tile([A, 512], BF16, tag="kT", bufs=2)
                relu_op(i % 2 == 1, kT[:, :sz], kps[:, :sz])
                kT_t[i] = kT

            def emit_relu_v(i):
                sz = SIZES[i]
                vT = kvp.tile([A, 512], BF16, tag="vT", bufs=3)
                relu_op(i % 2 == 0 and i not in (4, 6), vT[:, :sz],
                        vps_t[i][:, :sz])
                vT_t[i] = vT
                del vps_t[i]

            def emit_kt(i):
                sz = SIZES[i]
                ktp = ps_kt.tile([1, 512], F32, tag="ktp")
                nc.tensor.matmul(ktp[:, :sz], lhsT=up_sb[:, 0:1],
                                 rhs=kT_t[i][:, :sz], start=True, stop=True)
                ktp_t[i] = ktp
                del kT_t[i]

            def emit_exp(i):
                sz = SIZES[i]
                off = OFFS[i]
                nc.scalar.activation(
                    out=e_sb[0:1, off:off + sz], in_=ktp_t[i][:, :sz],
                    func=Exp, bias=0.0, scale=1.0,
                    accum_out=ssum[:, i:i + 1])
                del ktp_t[i]

            def emit_wb_stt(i):
                sz = SIZES[i]
                off = OFFS[i]
                wb = ps_wb.tile([128, 512], F32, tag="wb")
                nc.tensor.matmul(wb[:, :sz], lhsT=ones1,
                                 rhs=e_sb[0:1, off:off + sz],
                                 start=True, stop=True)
                prod = work.tile([A, 512], BF16, tag="prod")
                nc.vector.scalar_tensor_tensor(
                    out=prod[:, :sz], in0=vT_t[i][:, :sz], scalar=0.0,
                    in1=wb[:, :sz], op0=BYPASS, op1=MULT,
                    accum_out=partial[:, i:i + 1])
                del vT_t[i]

            for i in range(NCH):
                kps = emit_proj(i, ps_k, wk_sb, "kps")
                emit_relu_k(i, kps)
                if i >= 1:
                    emit_kt(i - 1)
                    emit_exp(i - 1)
                vps_t[i] = emit_proj(i, ps_v, wv_sb, "vps")
                emit_relu_v(i)
                if i >= 2:
                    emit_wb_stt(i - 2)
            emit_kt(NCH - 1)
            emit_exp(NCH - 1)
            emit_wb_stt(NCH - 2)
            emit_wb_stt(NCH - 1)

            # ---- unnormalized row + S; host divides and broadcasts.
            nc.vector.reduce_sum(out=out_pad[0:1, 1:2], in_=ssum, axis=AX,
                                 op=ADD)
            nc.vector.reduce_sum(out=out_pad[:, 0:1], in_=partial, axis=AX,
                                 op=ADD)
            nc.sync.dma_start(out=out[:, :], in_=out_pad)

    nc.finalize()
    return nc


def make_in_maps(inputs) -> list[dict]:
    f8 = ml_dtypes.float8_e4m3
    bf16 = ml_dtypes.bfloat16
    enc = np.asarray(inputs["encoder_outputs"], dtype=np.float32)
    Wk = np.asarray(inputs["Wk"], dtype=np.float32)
    Wv = np.asarray(inputs["Wv"], dtype=np.float32)
    Pu = np.asarray(inputs["Pu"], dtype=np.float32)
    pv = np.asarray(inputs["pv"], dtype=np.float32)

    u = (Pu @ pv).astype(np.float32) * INV_SQRT_A          # [A, 1]
    u_pad = np.zeros((A, 128), np.float32)
    u_pad[:, 0:1] = u
    u_pad = u_pad.astype(bf16)

    def wprep(w):  # [DE, A] -> [128, NDC*A], c-major per partition
        return np.ascontiguousarray(
            w.reshape(NDC, 128, A).transpose(1, 0, 2).reshape(128, -1)
        ).astype(f8)

    maps = []
    for b in range(B):
        encT = np.ascontiguousarray(enc[b].T).astype(f8)   # [DE, LE]
        m = {"wk": wprep(Wk), "wv": wprep(Wv), "u_pad": u_pad}
        for t, sz in enumerate(SIZES):
            blk = encT[:, OFFS[t]:OFFS[t] + sz]            # [DE, sz]
            m[f"enc{t}"] = np.ascontiguousarray(
                blk.reshape(NDC, 128, sz).transpose(1, 0, 2)
                .reshape(128, NDC * sz))
        maps.append(m)
    return maps


_NC_CACHE = None


def kernel(**inputs) -> np.ndarray:
    global _NC_CACHE
    in_maps = make_in_maps(inputs)
    if _NC_CACHE is None:
        _NC_CACHE = build_nc()
    res = run_bass_kernel_spmd(_NC_CACHE, in_maps, core_ids=list(range(B)))
    rows = []
    for b in range(B):
        o = np.asarray(res.results[b]["out"], dtype=np.float32)
        rows.append(o[:, 0] / o[0, 1])
    rows = np.stack(rows)                          # [B, A]
    return np.ascontiguousarray(
        np.broadcast_to(rows[:, None, :], (B, LD, A)).astype(np.float32)
    )


# revision 35
# speedup vs baseline: 1.0663x; 1.0382x over previous
"""Trainium2 Bass kernel for nn_Attention_24781961298297.

Math: scores[b,i,j] = (q_term[b,i] + k_term[b,j]) / sqrt(A).  Softmax over j
subtracts the row max, and q_term[b,i] is constant along j, so it cancels
exactly -- the attention weights are independent of i (and of the whole
decoder/q branch).  The output is one [A] vector per batch element,
broadcast over all Ld rows:

    kt[b,j] = relu(enc[b,j] @ Wk) @ (Pu @ pv)      (biases are zero)
    w[b]    = softmax(kt[b] / sqrt(A))
    row[b]  = w[b] @ relu(enc[b] @ Wv)
    out[b,i,:] = row[b]  for all i

Sharding: pure data-parallel over batch B=8 across the 8 cores (one batch
element per core, no collectives).

v7 notes (trace-driven, from six measured variants):
  * Fine-grained 10-chunk pipeline (256/512-token chunks) -- measured
    better wall-clock than every big-chunk variant (latency hiding beats
    per-op overhead savings on this machine).
  * enc ships fp8e4m3 in host-prepared piece-major blocks (8 pieces of
    512 tokens) whose per-partition rows are 2 KB contiguous -> 2 KB DMA
    descriptors.  v1's token-sliced layout gave 512 B descriptors and
    only ~88 GB/s; this layout measured ~300 GB/s aggregate.  Pieces
    alternate between the two HWDGE rings; Wk and Wv are separate 64 KB
    params, one per ring, ordered before the enc pieces.
  * Projections are fp8 DoubleRow matmuls (K=256/instruction, measured
    216 ns back-to-back at full clock, half the bf16 instruction count).
  * e row lives as [1, LE] (no [128, LE] memset -- saves 4.4 us of DVE);
    softmax-weight broadcast is a K=1 ones-matmul; kt is an M=1 matmul
    into a [1, 512] PSUM row.
  * Engine split per chunk: one relu on ACT and one on DVE (alternating
    K/V), exp on ACT, weighted-sum (scalar_tensor_tensor + accum) on
    DVE.  GpSimd cannot help (no PSUM access, no TensorScalar opcode).
  * Pipeline lags: ktp/exp one chunk behind the projections, wb/stt two
    behind -> no PE instruction waits on same-chunk ACT/DVE results.
    PSUM (1 bank each): kps(2) + vps(2) + ktp(2) + wb/warm(2) = 8.
"""

import numpy as np
import ml_dtypes

import concourse.bass as bass
import concourse.bacc as bacc
import concourse.tile as tile
from concourse import mybir
from concourse.bass_utils import run_bass_kernel_spmd

B, LE, LD = 8, 4096, 4096
DE, DD, A = 512, 512, 128

NDC = DE // 128                    # 4 DE subtiles
# one DMA piece per compute chunk; small first pieces so the first
# projection starts as early as possible
SIZES = [256, 256, 512, 512, 512, 512, 512, 512, 256, 256]
NCH = len(SIZES)
OFFS = [sum(SIZES[:i]) for i in range(NCH)]

INV_SQRT_A = float(1.0 / np.sqrt(np.float32(A)))

F32 = mybir.dt.float32
BF16 = mybir.dt.bfloat16
FP8 = mybir.dt.float8e4
Relu = mybir.ActivationFunctionType.Relu
Exp = mybir.ActivationFunctionType.Exp
AX = mybir.AxisListType.X
ADD = mybir.AluOpType.add
MAX = mybir.AluOpType.max
MULT = mybir.AluOpType.mult
BYPASS = mybir.AluOpType.bypass
DR = mybir.MatmulPerfMode.DoubleRow

N_WARM = 3


def build_nc() -> bass.Bass:
    nc = bacc.Bacc()

    enc_ps = [
        nc.declare_dram_parameter(f"enc{t}", [128, NDC * sz], FP8,
                                  isOutput=False)
        for t, sz in enumerate(SIZES)
    ]
    wk = nc.declare_dram_parameter("wk", [128, NDC * A], FP8, isOutput=False)
    wv = nc.declare_dram_parameter("wv", [128, NDC * A], FP8, isOutput=False)
    u_pad = nc.declare_dram_parameter("u_pad", [A, 128], BF16, isOutput=False)
    out = nc.declare_dram_parameter("out", [A, 128], F32, isOutput=True)

    with tile.TileContext(nc) as tc:
        with (
            tc.tile_pool(name="consts", bufs=1) as consts,
            tc.tile_pool(name="encpool", bufs=1) as encpool,
            tc.tile_pool(name="kvp", bufs=1) as kvp,
            tc.tile_pool(name="smallp", bufs=1) as smallp,
            tc.tile_pool(name="work", bufs=2) as work,
            tc.tile_pool(name="ps_k", bufs=2, space="PSUM") as ps_k,
            tc.tile_pool(name="ps_v", bufs=2, space="PSUM") as ps_v,
            tc.tile_pool(name="ps_kt", bufs=2, space="PSUM") as ps_kt,
            tc.tile_pool(name="ps_wb", bufs=2, space="PSUM") as ps_wb,
        ):
            # ---- DMAs split between the sync HWDGE ring and GpSimd's
            #      SWDGE (GpSimd is otherwise idle; configs on the ACT ring
            #      would cost ~667 ns each of ACT-queue time).  Weights
            #      first on each path, then pieces alternating.
            wk_sb = consts.tile([128, NDC, A], FP8, tag="wk")
            wv_sb = consts.tile([128, NDC, A], FP8, tag="wv")
            enc_sb = []
            for t, sz in enumerate(SIZES):
                et = encpool.tile([128, NDC, sz], FP8, tag=f"enc{t}",
                                  name=f"enc_sb{t}")
                enc_sb.append(et)
            up_sb = consts.tile([A, 128], BF16, tag="up")

            def piece_dma(eng, t):
                eng.dma_start(
                    out=enc_sb[t],
                    in_=enc_ps[t].rearrange("p (c j) -> p c j", c=NDC))

            # sync HWDGE: piece 0, Wk, then even pieces; GpSimd SWDGE:
            # Wv, piece 1, u, then odd pieces (FIFO per path matches
            # consumption order).
            piece_dma(nc.sync, 0)
            nc.sync.dma_start(out=wk_sb,
                              in_=wk.rearrange("p (c a) -> p c a", c=NDC))
            nc.gpsimd.dma_start(out=wv_sb,
                                in_=wv.rearrange("p (c a) -> p c a", c=NDC))
            piece_dma(nc.gpsimd, 1)
            nc.gpsimd.dma_start(out=up_sb, in_=u_pad[:, :])
            for t in range(2, NCH):
                piece_dma(nc.sync if t % 2 == 0 else nc.gpsimd, t)

            # ---- tiny SBUF constants + PE warm-up
            ones1 = consts.tile([1, 128], BF16, tag="ones1")
            nc.vector.memset(ones1, 1.0)
            wtile = consts.tile([1, 512], BF16, tag="wtile")
            nc.vector.memset(wtile, 0.5)
            for _ in range(N_WARM):
                warm_ps = ps_wb.tile([128, 512], F32, tag="wb")
                nc.tensor.matmul(warm_ps, lhsT=ones1, rhs=wtile,
                                 start=True, stop=True)

            e_sb = smallp.tile([1, LE], BF16, tag="e")
            ssum = smallp.tile([1, NCH], F32, tag="ssum")
            partial = smallp.tile([A, NCH], F32, tag="partial")
            out_pad = smallp.tile([A, 128], F32, tag="out_pad")
            nc.gpsimd.memset(out_pad, 0.0)

            vps_t = {}   # PSUM V-projection per chunk
            ktp_t = {}   # PSUM [1, sz] logits per chunk
            kT_t = {}    # SBUF relu'd K per chunk
            vT_t = {}    # SBUF relu'd V per chunk

            def emit_proj(i, pool, w_sb, tag):
                sz = SIZES[i]
                ps = pool.tile([128, 512], F32, tag=tag)
                for c in range(0, NDC, 2):
                    nc.tensor.matmul(
                        ps[:, :sz], lhsT=w_sb[:, c:c + 2, :],
                        rhs=enc_sb[i][:, c:c + 2, :],
                        start=(c == 0), stop=(c == NDC - 2),
                        perf_mode=DR,
                    )
                return ps

            def relu_op(on_act, dst, src):
                if on_act:
                    nc.scalar.activation(out=dst, in_=src, func=Relu,
                                         bias=0.0, scale=1.0)
                else:
                    nc.vector.tensor_scalar(out=dst, in0=src, scalar1=0.0,
                                            scalar2=None, op0=MAX)

            def emit_relu_k(i, kps):
                sz = SIZES[i]
                kT = kvp.tile([A, 512], BF16, tag="kT", bufs=2)
                relu_op(i % 2 == 1, kT[:, :sz], kps[:, :sz])
                kT_t[i] = kT

            def emit_relu_v(i):
                sz = SIZES[i]
                vT = kvp.tile([A, 512], BF16, tag="vT", bufs=3)
                relu_op(i % 2 == 0 and i not in (4, 6), vT[:, :sz],
                        vps_t[i][:, :sz])
                vT_t[i] = vT
                del vps_t[i]

            def emit_kt(i):
                sz = SIZES[i]
                ktp = ps_kt.tile([1, 512], F32, tag="ktp")
                nc.tensor.matmul(ktp[:, :sz], lhsT=up_sb[:, 0:1],
                                 rhs=kT_t[i][:, :sz], start=True, stop=True)
                ktp_t[i] = ktp
                del kT_t[i]

            def emit_exp(i):
                sz = SIZES[i]
                off = OFFS[i]
                nc.scalar.activation(
                    out=e_sb[0:1, off:off + sz], in_=ktp_t[i][:, :sz],
                    func=Exp, bias=0.0, scale=1.0,
                    accum_out=ssum[:, i:i + 1])
                del ktp_t[i]

            def emit_wb_stt(i):
                sz = SIZES[i]
                off = OFFS[i]
                wb = ps_wb.tile([128, 512], F32, tag="wb")
                nc.tensor.matmul(wb[:, :sz], lhsT=ones1,
                                 rhs=e_sb[0:1, off:off + sz],
                                 start=True, stop=True)
                prod = work.tile([A, 512], BF16, tag="prod")
                nc.vector.scalar_tensor_tensor(
                    out=prod[:, :sz], in0=vT_t[i][:, :sz], scalar=0.0,
                    in1=wb[:, :sz], op0=BYPASS, op1=MULT,
                    accum_out=partial[:, i:i + 1])
                del vT_t[i]

            for i in range(NCH):
                kps = emit_proj(i, ps_k, wk_sb, "kps")
                emit_relu_k(i, kps)
                if i >= 1:
                    emit_kt(i - 1)
                    emit_exp(i - 1)
                vps_t[i] = emit_proj(i, ps_v, wv_sb, "vps")
                emit_relu_v(i)
                if i >= 2:
                    emit_wb_stt(i - 2)
            emit_kt(NCH - 1)
            emit_exp(NCH - 1)
            emit_wb_stt(NCH - 2)
            emit_wb_stt(NCH - 1)

            # ---- unnormalized row + S; host divides and broadcasts.
            nc.vector.reduce_sum(out=out_pad[0:1, 1:2], in_=ssum, axis=AX,
                                 op=ADD)
            nc.vector.reduce_sum(out=out_pad[:, 0:1], in_=partial, axis=AX,
                                 op=ADD)
            nc.sync.dma_start(out=out[:, :], in_=out_pad)

    nc.finalize()
    return nc


def make_in_maps(inputs) -> list[dict]:
    f8 = ml_dtypes.float8_e4m3
    bf16 = ml_dtypes.bfloat16
    enc = np.asarray(inputs["encoder_outputs"], dtype=np.float32)
    Wk = np.asarray(inputs["Wk"], dtype=np.float32)
    Wv = np.asarray(inputs["Wv"], dtype=np.float32)
    Pu = np.asarray(inputs["Pu"], dtype=np.float32)
    pv = np.asarray(inputs["pv"], dtype=np.float32)

    u = (Pu @ pv).astype(np.float32) * INV_SQRT_A          # [A, 1]
    u_pad = np.zeros((A, 128), np.float32)
    u_pad[:, 0:1] = u
    u_pad = u_pad.astype(bf16)

    def wprep(w):  # [DE, A] -> [128, NDC*A], c-major per partition
        return np.ascontiguousarray(
            w.reshape(NDC, 128, A).transpose(1, 0, 2).reshape(128, -1)
        ).astype(f8)

    maps = []
    for b in range(B):
        encT = np.ascontiguousarray(enc[b].T).astype(f8)   # [DE, LE]
        m = {"wk": wprep(Wk), "wv": wprep(Wv), "u_pad": u_pad}
        for t, sz in enumerate(SIZES):
            blk = encT[:, OFFS[t]:OFFS[t] + sz]            # [DE, sz]
            m[f"enc{t}"] = np.ascontiguousarray(
                blk.reshape(NDC, 128, sz).transpose(1, 0, 2)
                .reshape(128, NDC * sz))
        maps.append(m)
    return maps


_NC_CACHE = None


def kernel(**inputs) -> np.ndarray:
    global _NC_CACHE
    in_maps = make_in_maps(inputs)
    if _NC_CACHE is None:
        _NC_CACHE = build_nc()
    res = run_bass_kernel_spmd(_NC_CACHE, in_maps, core_ids=list(range(B)))
    rows = []
    for b in range(B):
        o = np.asarray(res.results[b]["out"], dtype=np.float32)
        rows.append(o[:, 0] / o[0, 1])
    rows = np.stack(rows)                          # [B, A]
    return np.ascontiguousarray(
        np.broadcast_to(rows[:, None, :], (B, LD, A)).astype(np.float32)
    )
